# revision 2
# baseline (speedup 1.0000x reference)
"""Trainium2 Bass kernel for nn_NodeEmbedding (GNN message passing).

Strategy (edge sharding, no collectives), v2:
  - Host: sort edges by destination row; split the 50k nodes across 8 cores
    (6250 each), group edges into 128-node windows padded to a uniform
    per-window chunk count so the SPMD program is static.
  - The neighbor-embedding rows are looked up ON DEVICE instead of being
    host-gathered and uploaded (the baseline's 39MB/core `nrows` input):
    a [1,EP] bf16 vector of neighbor types `zc` is DMA-broadcast across
    partitions, compared against a partition-index constant to build the
    one-hot S_T[t,e] = (zc[e]==t), and nr[e,h] = S_T.T @ nemb on the PE.
  - Cutoff C and the projection bias are folded into an augmented transposed
    edge-feature matrix eaT [65, E] (bf16): W = eaT.T @ P65 on the PE.
  - msg = W * nr (DVE); segment_sum via one-hot(node-in-window) matmul:
    aggT[h, n] += msg[e,h].T @ oh[e,n].
  - combine: out[n,o] = aggT.T @ W2.T + (atom_emb@W1.T + b)[z[n]]; the second
    table is row-gathered per 128-node window with indirect DMA.
"""

import os
import sys

import numpy as np

for p in ("/opt/trn_rl_repo",):
    if p not in sys.path and os.path.isdir(p):
        sys.path.insert(0, p)

import ml_dtypes

N_NODES = 50000
N_EDGES = 800000
H = 128
RBF = 64
CUTOFF = 5.0
MAX_Z = 100
NT = MAX_Z + 1  # 101 types
NCORES = 8
NPC = N_NODES // NCORES  # 6250 nodes per core
WIN = 128
NW = 52  # windows per core (52*128 = 6656 >= 6250; chosen so NW*CW % SC == 0)
NLP = NW * WIN  # 6656 padded nodes per core
SC = 8  # chunks (of 128 edges) per supertile

TRACE = False
LAST_PERF = {}


def _prep(z, edge_index, edge_dist, edge_attr):
    """Sort/shard/pad edges; returns per-core arrays + layout constants."""
    f32 = np.float32
    bf16 = ml_dtypes.bfloat16
    row = np.asarray(edge_index[0], dtype=np.int64)
    col = np.asarray(edge_index[1], dtype=np.int64)
    d = np.asarray(edge_dist, dtype=f32)
    C = (0.5 * (np.cos(np.pi * d / CUTOFF) + 1.0)).astype(f32) * (d < CUTOFF)
    ea = np.asarray(edge_attr, dtype=f32)
    eaC = np.empty((N_EDGES, RBF + 1), dtype=f32)
    eaC[:, :RBF] = ea * C[:, None]
    eaC[:, RBF] = C
    zc = np.asarray(z, dtype=np.int64)[col].astype(np.int32)

    order = np.argsort(row, kind="stable")
    row_s = row[order]
    eaC_s = eaC[order]
    zc_s = zc[order]

    core_of = row_s // NPC
    local = row_s - core_of * NPC
    w_of = local // WIN
    rel = (local - w_of * WIN).astype(f32)

    cw_key = core_of * NW + w_of
    counts = np.bincount(cw_key, minlength=NCORES * NW)
    ewmax = int(counts.max())
    CW = (ewmax + 127) // 128
    CW += CW % 2  # NW*CW % SC == 0 needs CW even (52*CW % 8)
    EW = CW * 128
    CH = NW * CW
    EP = CH * 128

    starts = np.zeros(NCORES * NW + 1, dtype=np.int64)
    np.cumsum(counts, out=starts[1:])
    off_in_win = np.arange(len(row_s), dtype=np.int64) - starts[cw_key]
    dest = w_of * EW + off_in_win  # within-core flat slot

    eaT = np.zeros((NCORES, RBF + 1, EP), dtype=bf16)
    zcr = np.zeros((NCORES, EP), dtype=bf16)
    rloc = np.zeros((NCORES, EP), dtype=bf16)
    for i in range(NCORES):
        m = core_of == i
        eaT[i][:, dest[m]] = eaC_s[m].T.astype(bf16)
        zcr[i][dest[m]] = zc_s[m].astype(bf16)
        rloc[i][dest[m]] = rel[m].astype(bf16)
    # rloc: [EP] -> [128, CH] with flat = c*128 + p
    rloc = np.ascontiguousarray(rloc.reshape(NCORES, CH, 128).transpose(0, 2, 1))
    return eaT, zcr, rloc, CW, CH, EP


def _split_waits(nc):
    """Hoist excess sem-waits onto same-engine NoOps.

    The axon walrus toolchain accepts very few sync-wait slots per
    instruction; a NoOp issued just before on the same engine satisfies the
    wait in program order instead.
    """
    import concourse.mybir as mybir

    k = 0
    for fn in nc.m.functions:
        for bb in fn.blocks:
            il = bb.instructions
            i = 0
            while i < len(il):
                inst = il[i]
                si = inst.sync_info
                if si is not None and si.on_wait and len(si.on_wait) > 1:
                    waits = list(si.on_wait)
                    keep, excess = waits[:1], waits[1:]
                    for w in excess:
                        nop = mybir.InstNoOp(name=f"wsplit-{k}")
                        k += 1
                        nop.engine = inst.engine
                        nop.sync_info = mybir.SyncInfo(
                            on_wait=[w], on_update=[]
                        )
                        il.insert(i, nop)
                        i += 1
                    inst.sync_info = mybir.SyncInfo(
                        on_wait=keep, on_update=list(si.on_update or [])
                    )
                i += 1


def _build_program(CW, CH, EP):
    import concourse.bass as bass
    import concourse.mybir as mybir
    import concourse.tile as tile

    f32 = mybir.dt.float32
    bf16 = mybir.dt.bfloat16
    i32 = mybir.dt.int32
    NST = CH // SC
    SE = SC * 128  # edges per supertile

    nc = bass.Bass()
    ea_d = nc.dram_tensor("eaT", [RBF + 1, EP], bf16, kind="ExternalInput")
    zcr_d = nc.dram_tensor("zcr", [1, EP], bf16, kind="ExternalInput")
    rloc_d = nc.dram_tensor("rloc", [128, CH], bf16, kind="ExternalInput")
    zwin_d = nc.dram_tensor("zwin", [128, NW], i32, kind="ExternalInput")
    t1r_d = nc.dram_tensor("t1r", [NT, H], f32, kind="ExternalInput")
    w2_d = nc.dram_tensor("w2t", [128, H], bf16, kind="ExternalInput")
    p65_d = nc.dram_tensor("p65", [RBF + 1, H], bf16, kind="ExternalInput")
    nemb_d = nc.dram_tensor("nembp", [128, H], bf16, kind="ExternalInput")
    titer_d = nc.dram_tensor("titer", [128, 1], bf16, kind="ExternalInput")
    iota_d = nc.dram_tensor("iota", [128, SE], bf16, kind="ExternalInput")
    out_d = nc.dram_tensor("outT", [NLP, H], bf16, kind="ExternalOutput")

    with tile.TileContext(nc) as tc:
        with (
            tc.tile_pool(name="const", bufs=1) as cp,
            tc.tile_pool(name="ea", bufs=3) as eap,
            tc.tile_pool(name="zb", bufs=2) as zbp,
            tc.tile_pool(name="st", bufs=2) as stp,
            tc.tile_pool(name="wb", bufs=2) as wbp,
            tc.tile_pool(name="msg", bufs=2) as msp,
            tc.tile_pool(name="oh", bufs=2) as ohp,
            tc.tile_pool(name="wind", bufs=2) as wnp,
            tc.tile_pool(name="wps", bufs=1, space="PSUM") as wps,
            tc.tile_pool(name="nrps", bufs=2, space="PSUM") as nrps,
            tc.tile_pool(name="aggp", bufs=1, space="PSUM") as aggp,
            tc.tile_pool(name="outp", bufs=1, space="PSUM") as outp,
        ):
            rloc_t = cp.tile([128, CH], bf16, tag="rloc")
            nc.sync.dma_start(rloc_t[:], rloc_d[:])
            zwin_t = cp.tile([128, NW], i32, tag="zwin")
            nc.sync.dma_start(zwin_t[:], zwin_d[:])
            w2_t = cp.tile([128, H], bf16, tag="w2")
            nc.sync.dma_start(w2_t[:], w2_d[:])
            p65_t = cp.tile([RBF + 1, H], bf16, tag="p65")
            nc.sync.dma_start(p65_t[:], p65_d[:])
            nemb_t = cp.tile([128, H], bf16, tag="nemb")
            nc.sync.dma_start(nemb_t[:], nemb_d[:])
            titer_t = cp.tile([128, 1], bf16, tag="titer")
            nc.sync.dma_start(titer_t[:], titer_d[:])
            iota_t = cp.tile([128, SC, 128], bf16, tag="iota")
            nc.sync.dma_start(iota_t[:].rearrange("p s j -> p (s j)"), iota_d[:])

            tc.strict_bb_all_engine_barrier()

            agg = [None]
            for st in range(NST):
                e0 = st * SE
                ea_t = eap.tile([RBF + 1, SE], bf16, tag="ea")
                nc.sync.dma_start(ea_t[:], ea_d[:, e0 : e0 + SE])
                zbc = zbp.tile([128, SE], bf16, tag="zb")
                nc.sync.dma_start(
                    zbc[:], zcr_d[:, e0 : e0 + SE].broadcast_to((128, SE))
                )
                st_t = stp.tile([128, SE], bf16, tag="st")
                nc.vector.tensor_tensor(
                    st_t[:],
                    zbc[:],
                    titer_t[:].broadcast_to((128, SE)),
                    op=mybir.AluOpType.is_equal,
                )
                wt = wps.tile([128, SE], f32, tag="w")
                for j in range(SC):
                    nc.tensor.matmul(
                        wt[:, j * 128 : (j + 1) * 128],
                        ea_t[:, j * 128 : (j + 1) * 128],
                        p65_t[:],
                        start=True,
                        stop=True,
                    )
                nrt = nrps.tile([128, SE], f32, tag="nr")
                for j in range(SC):
                    nc.tensor.matmul(
                        nrt[:, j * 128 : (j + 1) * 128],
                        st_t[:, j * 128 : (j + 1) * 128],
                        nemb_t[:],
                        start=True,
                        stop=True,
                    )
                # ACT evicts W PSUM as bf16; DVE multiplies it with nr (PSUM)
                wb = wbp.tile([128, SE], bf16, tag="wb")
                nc.scalar.copy(wb[:], wt[:])
                ms = msp.tile([128, SE], bf16, tag="ms")
                nc.vector.tensor_tensor(
                    ms[:], wb[:], nrt[:], op=mybir.AluOpType.mult
                )
                oh = ohp.tile([128, SC, 128], bf16, tag="oh")
                rl = rloc_t[:, st * SC : (st + 1) * SC].unsqueeze(-1)
                nc.vector.tensor_tensor(
                    oh[:],
                    iota_t[:],
                    rl.broadcast_to((128, SC, 128)),
                    op=mybir.AluOpType.is_equal,
                )
                for j in range(SC):
                    c = st * SC + j
                    w = c // CW
                    if c % CW == 0:
                        agg[0] = aggp.tile(
                            [128, 128], f32, tag="agg", name=f"agg{w}"
                        )
                    nc.tensor.matmul(
                        agg[0][:],
                        ms[:, j * 128 : (j + 1) * 128],
                        oh[:, j, :],
                        start=(c % CW == 0),
                        stop=(c % CW == CW - 1),
                    )
                    if c % CW == CW - 1:
                        ag = wnp.tile([128, 128], bf16, tag="ag")
                        nc.vector.tensor_copy(ag[:], agg[0][:])
                        ot = outp.tile([128, 128], f32, tag="ot")
                        nc.tensor.matmul(
                            ot[:], ag[:], w2_t[:], start=True, stop=True
                        )
                        t1c = wnp.tile([128, 128], f32, tag="t1c")
                        nc.gpsimd.indirect_dma_start(
                            out=t1c[:],
                            out_offset=None,
                            in_=t1r_d[:],
                            in_offset=bass.IndirectOffsetOnAxis(
                                ap=zwin_t[:, w : w + 1], axis=0
                            ),
                        )
                        ob = wnp.tile([128, 128], bf16, tag="ob")
                        nc.vector.tensor_tensor(
                            ob[:], ot[:], t1c[:], op=mybir.AluOpType.add
                        )
                        nc.sync.dma_start(
                            out_d[w * 128 : (w + 1) * 128, :], ob[:]
                        )
    _split_waits(nc)
    return nc


def kernel(z, edge_index, edge_dist, edge_attr, atom_emb, neighbor_emb,
           proj_W, proj_b, comb_W, comb_b):
    from concourse.bass_utils import run_bass_kernel_spmd

    f32 = np.float32
    bf16 = ml_dtypes.bfloat16
    z = np.asarray(z)
    atom_emb = np.asarray(atom_emb, dtype=f32)
    neighbor_emb = np.asarray(neighbor_emb, dtype=f32)
    proj_W = np.asarray(proj_W, dtype=f32)
    proj_b = np.asarray(proj_b, dtype=f32)
    comb_W = np.asarray(comb_W, dtype=f32)
    comb_b = np.asarray(comb_b, dtype=f32)

    eaT, zcr, rloc, CW, CH, EP = _prep(z, edge_index, edge_dist, edge_attr)
    nc = _build_program(CW, CH, EP)

    T1 = (atom_emb @ comb_W[:, :H].T + comb_b).astype(f32)  # [101, 128]
    w2t = np.ascontiguousarray(comb_W[:, H:].T).astype(bf16)  # [h_in, out]
    p65 = np.concatenate([proj_W.T, proj_b[None, :]], axis=0).astype(bf16)
    nembp = np.zeros((128, H), dtype=bf16)
    nembp[:NT] = neighbor_emb.astype(bf16)
    titer = np.arange(128, dtype=f32).astype(bf16)[:, None]
    iota = np.ascontiguousarray(
        np.tile(np.arange(128, dtype=f32)[None, :], (128, SC))
    ).astype(bf16)

    zpad = np.zeros((NCORES, NLP), dtype=np.int32)
    zarr = np.asarray(z, dtype=np.int32)
    for i in range(NCORES):
        zpad[i, :NPC] = zarr[i * NPC : (i + 1) * NPC]
    # zwin[p, w] = z of node w*128+p
    zwin = np.ascontiguousarray(
        zpad.reshape(NCORES, NW, 128).transpose(0, 2, 1)
    )

    in_maps = []
    for i in range(NCORES):
        in_maps.append(
            {
                "eaT": np.ascontiguousarray(eaT[i]),
                "zcr": zcr[i][None, :],
                "rloc": rloc[i],
                "zwin": zwin[i],
                "t1r": T1,
                "w2t": w2t,
                "p65": p65,
                "nembp": nembp,
                "titer": titer,
                "iota": iota,
            }
        )

    try:
        res = run_bass_kernel_spmd(
            nc, in_maps, core_ids=list(range(NCORES)), trace=TRACE
        )
    except Exception:
        # one retry: the axon worker occasionally reports a stale
        # "unrecoverable" state from a previous process's crash
        res = run_bass_kernel_spmd(
            nc, in_maps, core_ids=list(range(NCORES)), trace=TRACE
        )
    LAST_PERF.clear()
    LAST_PERF.update(
        exec_time_ns=res.exec_time_ns,
        mean_exec_time_ns=res.mean_exec_time_ns,
        trace=getattr(res, "instructions_and_trace", None),
        layout=(CW, CH, EP),
    )

    out = np.empty((N_NODES, H), dtype=f32)
    for i in range(NCORES):
        out[i * NPC : (i + 1) * NPC] = res.results[i]["outT"][:NPC].astype(f32)
    return out
